# revision 23
# baseline (speedup 1.0000x reference)
"""Trainium2 Bass kernel for nn_ChunkedAttention (B=4, T=4096, D=1024, H=16, dh=64).

Sharding: 8 cores = 4 batches x 2 head-groups (8 heads each). Host sums the
two partial out-projections per batch.

Datapath (per core):
  - Host passes x^T, so projections need no on-device transposes.
  - Q^T/K^T/V projected with fp32r matmuls, stored resident in SBUF as bf16.
  - Scores: per (qg, hp, kt) two K=64 bf16 matmuls packed into PE quadrants
    via tile_position -> PSUM f32 [k, q], one tile per head (j).
  - exp per (kt, j): j=0 exact Exp on ACT; j=1 Schraudolph fast-exp on DVE
    (i16 = rne(A*s + B), bitcast to bf16; ~1.8% rel RMS), with every 8th
    k-tile's j=1 on ACT to balance engine load. Per-j split halves the
    per-tile exp latency so the 2-ktile-deep AV software pipeline hides it.
  - AV: [dh+1, q] bf16 matmuls (lhsT = V with a ones column) accumulated
    over kt in PSUM; row 64 is the softmax denominator.
  - Normalize: copy av out of PSUM on ACT (frees the bank early), then
    reciprocal (DVE) -> DMA hop to partition 0 (partition_broadcast ucode
    reads p0) -> partition_broadcast (Pool) -> multiply (DVE). Head j=1's
    rows hop to partitions 64-127 via an SBUF->SBUF DMA.
  - Out-proj: K=128 bf16 matmuls over head-pair dims, accumulate over hp,
    yt staging copy on ACT, DMA -> y.
  - build_nc(repeat=n) emits the whole body n times in one NEFF; test.py
    uses the repeat-count slope to cancel dispatch overhead when timing.
"""

import os
import sys

import numpy as np

for _p in ("/opt/trn_rl_repo",):
    if _p not in sys.path and os.path.isdir(_p):
        sys.path.insert(0, _p)

import concourse.bass as bass
import concourse.mybir as mybir
import concourse.tile as tile
from concourse.bacc import Bacc
from concourse.bass_utils import run_bass_kernel_spmd

F32 = mybir.dt.float32
F32R = mybir.dt.float32r
BF16 = mybir.dt.bfloat16
I16 = mybir.dt.int16
EXP = mybir.ActivationFunctionType.Exp
MULT = mybir.AluOpType.mult
ADD = mybir.AluOpType.add

B, T, D = 4, 4096, 1024
HG = 512          # head-group width per core (8 heads x 64)
NH, DH = 8, 64    # heads per core, head dim
NPAIR = 4         # head pairs per core
QG = 512          # query-group width
NQG = T // QG     # 8
NKT = T // 128    # 32 k-tiles
NTC = T // 512    # 8 T-chunks in projection phase
NDC = D // 128    # 8 d_model chunks
SCALE = 1.0 / np.sqrt(DH)  # 0.125

# Schraudolph fast-exp in the bf16 bit domain: i16 = rne(A*logit + B16),
# bitcast i16 -> bf16 ~= exp(logit). Constants tuned for min RMS (~1.8%).
A16 = 184.6650390625            # 2**7 * log2(e)
B16 = 127.0 * 128.0 - 7.5
SCH_SCALE = A16 * SCALE         # folds the 1/sqrt(dh) into the affine


def build_nc(debug=None, repeat=1):
    nc = Bacc()
    xT_d = nc.dram_tensor("xT", [D, T], F32, kind="ExternalInput")
    wqT_d = nc.dram_tensor("wqT", [D, HG], F32, kind="ExternalInput")
    wkT_d = nc.dram_tensor("wkT", [D, HG], F32, kind="ExternalInput")
    wvT_d = nc.dram_tensor("wvT", [D, HG], F32, kind="ExternalInput")
    woT_d = nc.dram_tensor("woT", [HG, D], BF16, kind="ExternalInput")
    tri_d = nc.dram_tensor("tri", [128, 128], BF16, kind="ExternalInput")
    y_d = nc.dram_tensor("y", [T, D], F32, kind="ExternalOutput")

    with tile.TileContext(nc) as tc:
        with (
            tc.tile_pool(name="const", bufs=1) as pconst,
            tc.tile_pool(name="psS", bufs=4, space="PSUM") as psS,
            tc.tile_pool(name="psA", bufs=2, space="PSUM") as psA,
            tc.tile_pool(name="psO", bufs=2, space="PSUM") as psO,
        ):
            qt_sb = pconst.tile([128, NPAIR, T], BF16, tag="qt")
            kt_sb = pconst.tile([128, NPAIR, T], BF16, tag="kt")
            v_sb = pconst.tile([128, NKT, NH, DH + 1], BF16, tag="v")
            tri_sb = pconst.tile([128, 128], BF16, tag="tri")
            nc.sync.dma_start(tri_sb[:], tri_d[:])
            nc.gpsimd.memset(v_sb[:, :, :, DH : DH + 1], 1.0)

            for _rep in range(repeat):
                _emit_body(nc, tc, qt_sb, kt_sb, v_sb, tri_sb,
                           xT_d, wqT_d, wkT_d, wvT_d, woT_d, y_d,
                           psS, psA, psO)
    nc.compile()
    return nc


def _emit_body(nc, tc, qt_sb, kt_sb, v_sb, tri_sb,
               xT_d, wqT_d, wkT_d, wvT_d, woT_d, y_d, psS, psA, psO):
    if True:
        if True:
            # ---------- phase 1: Q^T/K^T/V projections from host-side x^T ----
            with (
                tc.tile_pool(name="pw", bufs=1) as pw,
                tc.tile_pool(name="pxin", bufs=2) as pxin,
            ):
                wq_sb = pw.tile([128, NDC, HG], F32R, tag="wq")
                wk_sb = pw.tile([128, NDC, HG], F32R, tag="wk")
                wv_sb = pw.tile([128, NDC, HG], F32R, tag="wv")
                xT_r = xT_d.bitcast(F32R).rearrange("(dc p) t -> p dc t", p=128)
                xt0 = pxin.tile([128, NDC, QG], F32R, tag="xt")
                for dc in range(NDC):
                    nc.sync.dma_start(xt0[:, dc, :], xT_r[:, dc, 0:QG])
                for dc in range(NDC):
                    nc.sync.dma_start(
                        wq_sb[:, dc, :],
                        wqT_d.bitcast(F32R).rearrange(
                            "(dc p) h -> p dc h", p=128
                        )[:, dc, :],
                    )
                for dc in range(NDC):
                    nc.sync.dma_start(
                        wk_sb[:, dc, :],
                        wkT_d.bitcast(F32R).rearrange(
                            "(dc p) h -> p dc h", p=128
                        )[:, dc, :],
                    )
                for dc in range(NDC):
                    nc.sync.dma_start(
                        wv_sb[:, dc, :],
                        wvT_d.bitcast(F32R).rearrange(
                            "(dc p) h -> p dc h", p=128
                        )[:, dc, :],
                    )
                for tcn in range(NTC):
                    if tcn == 0:
                        xt = xt0
                    else:
                        xt = pxin.tile([128, NDC, QG], F32R, tag="xt")
                        for dc in range(NDC):
                            nc.sync.dma_start(
                                xt[:, dc, :],
                                xT_r[:, dc, tcn * QG : (tcn + 1) * QG],
                            )
                    # Q^T, K^T -> [hg, t] bf16 resident
                    for hp in range(NPAIR):
                        pq = psS.tile([128, QG], F32, tag="s", name="pq")
                        for dc in range(NDC):
                            nc.tensor.matmul(
                                pq[:],
                                wq_sb[:, dc, hp * 128 : (hp + 1) * 128],
                                xt[:, dc, :],
                                start=(dc == 0),
                                stop=(dc == NDC - 1),
                            )
                        nc.vector.tensor_copy(
                            qt_sb[:, hp, tcn * QG : (tcn + 1) * QG], pq[:]
                        )
                        pk = psS.tile([128, QG], F32, tag="s", name="pk")
                        for dc in range(NDC):
                            nc.tensor.matmul(
                                pk[:],
                                wk_sb[:, dc, hp * 128 : (hp + 1) * 128],
                                xt[:, dc, :],
                                start=(dc == 0),
                                stop=(dc == NDC - 1),
                            )
                        nc.vector.tensor_copy(
                            kt_sb[:, hp, tcn * QG : (tcn + 1) * QG], pk[:]
                        )
                    # V -> [t, h, dh] bf16 resident (ones column at dh)
                    for ts in range(4):
                        pv = psO.tile([128, QG], F32, tag="op")
                        for dc in range(NDC):
                            nc.tensor.matmul(
                                pv[:],
                                xt[:, dc, ts * 128 : (ts + 1) * 128],
                                wv_sb[:, dc, :],
                                start=(dc == 0),
                                stop=(dc == NDC - 1),
                            )
                        nc.scalar.copy(
                            v_sb[:, tcn * 4 + ts, :, 0:DH],
                            pv.rearrange("p (h d) -> p h d", h=NH),
                        )

            # ---------- phase 2: attention + out-projection ----------
            with (
                tc.tile_pool(name="p2", bufs=1) as p2,
                tc.tile_pool(name="pet", bufs=6) as pet,
                tc.tile_pool(name="pnm", bufs=4) as pnm,
                tc.tile_pool(name="prc", bufs=3) as prc,
                tc.tile_pool(name="pmg", bufs=3) as pmg,
                tc.tile_pool(name="pyt", bufs=3) as pyt,
                tc.tile_pool(name="podt", bufs=3) as podt,
            ):
                wo_sb = p2.tile([128, NPAIR, D], BF16, tag="wo")
                nc.sync.dma_start(
                    wo_sb[:], woT_d.rearrange("(hp p) e -> p hp e", p=128)
                )
                def emit_outproj(oqg, omrgT, part):
                    # part 0..3 -> (half, qc-pair) chunk of the 8 op tiles
                    half = part // 2
                    for qc in (2 * (part % 2), 2 * (part % 2) + 1):
                        op = psO.tile([128, QG], F32, tag="op")
                        for hp in range(NPAIR):
                            nc.tensor.matmul(
                                op[:],
                                omrgT[:, hp, qc * 128 : (qc + 1) * 128],
                                wo_sb[:, hp, half * QG : (half + 1) * QG],
                                start=(hp == 0),
                                stop=(hp == NPAIR - 1),
                            )
                        yt = pyt.tile([128, QG], F32, tag="yt")
                        (nc.scalar.copy if qc % 2 == 0 else nc.vector.tensor_copy)(
                            yt[:], op[:]
                        )
                        nc.sync.dma_start(
                            y_d[
                                oqg * QG + qc * 128 : oqg * QG + (qc + 1) * 128,
                                half * QG : (half + 1) * QG,
                            ],
                            yt[:],
                        )

                prev = None
                for qg in range(NQG):
                    ktmax = 4 * (qg + 1)
                    mrgT = pmg.tile([128, NPAIR, QG], BF16, tag="mrgT")
                    for hp in range(NPAIR):
                        if prev is not None:
                            emit_outproj(prev[0], prev[1], hp)
                        avs = [
                            psA.tile([128, QG], F32, tag="av", name=f"av{j}")
                            for j in range(2)
                        ]

                        def emit_av(kt, dlt, e_t):
                            # V cols [data(64), ones] -> rows 0-63 data, 64 den
                            for j in range(2):
                                nc.tensor.matmul(
                                    avs[j][0 : DH + 1, dlt:],
                                    v_sb[:, kt, 2 * hp + j, :],
                                    e_t[:, j, dlt:],
                                    start=(kt == 0),
                                    stop=(kt == ktmax - 1),
                                )

                        pending = []
                        for kt in range(ktmax):
                            diag = kt - 4 * qg
                            dlt = 128 * diag if diag > 0 else 0
                            e_t = pet.tile([128, 2, QG], BF16, tag="e")
                            for j in range(2):
                                s_t = psS.tile(
                                    [128, QG], F32, tag="s", name=f"s{j}"
                                )
                                nc.tensor.matmul(
                                    s_t[:, dlt:],
                                    kt_sb[
                                        64 * j : 64 * (j + 1),
                                        hp,
                                        kt * 128 : (kt + 1) * 128,
                                    ],
                                    qt_sb[
                                        64 * j : 64 * (j + 1),
                                        hp,
                                        qg * QG + dlt : (qg + 1) * QG,
                                    ],
                                    start=True,
                                    stop=True,
                                    tile_position=(64 * j, 0),
                                )
                                # j=0 -> exact exp on ACT; j=1 -> fast exp on
                                # DVE/Pool (alternating by kt)
                                if j == 0:
                                    nc.scalar.activation(
                                        e_t[:, 0, dlt:],
                                        s_t[:, dlt:],
                                        EXP,
                                        scale=SCALE,
                                    )
                                elif kt % 8 == 7:
                                    nc.scalar.activation(
                                        e_t[:, 1, dlt:],
                                        s_t[:, dlt:],
                                        EXP,
                                        scale=SCALE,
                                    )
                                else:
                                    nc.vector.tensor_scalar(
                                        e_t.bitcast(I16)[:, 1, dlt:],
                                        s_t[:, dlt:],
                                        SCH_SCALE,
                                        B16,
                                        MULT,
                                        ADD,
                                    )
                                if diag >= 0:
                                    nc.vector.tensor_tensor(
                                        e_t[:, j, dlt : dlt + 128],
                                        e_t[:, j, dlt : dlt + 128],
                                        tri_sb[:],
                                        MULT,
                                    )
                            pending.append((kt, dlt, e_t))
                            if len(pending) > 2:
                                emit_av(*pending.pop(0))
                        while pending:
                            emit_av(*pending.pop(0))

                        # normalize: copy av out of PSUM fast (ACT) so the
                        # banks free early; reciprocal both dens into one row
                        # tile; one DMA hop to partition 0 (partition_broadcast
                        # ucode reads p0) + one broadcast covers both heads.
                        craws = []
                        rr = prc.tile([128, 2, QG], F32, tag="rr")
                        for j in range(2):
                            craw = prc.tile(
                                [128, QG], F32, tag="craw", name=f"craw{j}"
                            )
                            nc.scalar.copy(
                                craw[0 : DH + 1, :], avs[j][0 : DH + 1, :]
                            )
                            craws.append(craw)
                            nc.vector.reciprocal(
                                rr[DH : DH + 1, j, :], craw[DH : DH + 1, :]
                            )
                        rs0 = prc.tile([1, 2, QG], F32, tag="rs0")
                        nc.sync.dma_start(rs0[:], rr[DH : DH + 1, :, :])
                        rb = pnm.tile([64, 2, QG], F32, tag="rb")
                        nc.gpsimd.partition_broadcast(rb[:], rs0[:])
                        nc.vector.tensor_tensor(
                            mrgT[0:DH, hp, :],
                            craws[0][0:DH, :],
                            rb[:, 0, :],
                            MULT,
                        )
                        odt = podt.tile([64, QG], BF16, tag="odt")
                        nc.vector.tensor_tensor(
                            odt[:], craws[1][0:DH, :], rb[:, 1, :], MULT
                        )
                        nc.sync.dma_start(mrgT[DH:128, hp, :], odt[:])

                    prev = (qg, mrgT)
                for part in range(4):
                    emit_outproj(prev[0], prev[1], part)


_NC_CACHE = None


def _get_nc():
    global _NC_CACHE
    if _NC_CACHE is None:
        _NC_CACHE = build_nc()
    return _NC_CACHE


def make_in_maps(x, Wq, Wk, Wv, Wo):
    import ml_dtypes

    bf = ml_dtypes.bfloat16
    x = np.asarray(x, dtype=np.float32)
    Wq = np.asarray(Wq, dtype=np.float32)
    Wk = np.asarray(Wk, dtype=np.float32)
    Wv = np.asarray(Wv, dtype=np.float32)
    Wo = np.asarray(Wo, dtype=np.float32)
    tri = np.triu(np.ones((128, 128), dtype=np.float32)).astype(bf)
    in_maps = []
    for c in range(8):
        b, g = divmod(c, 2)
        rows = slice(HG * g, HG * (g + 1))
        in_maps.append(
            {
                "xT": np.ascontiguousarray(x[b].T),
                "wqT": np.ascontiguousarray(Wq[rows].T),
                "wkT": np.ascontiguousarray(Wk[rows].T),
                "wvT": np.ascontiguousarray(Wv[rows].T),
                "woT": np.ascontiguousarray(Wo[:, rows].T).astype(bf),
                "tri": tri,
            }
        )
    return in_maps


def run(x, Wq, Wk, Wv, Wo, trace=False, **spmd_kwargs):
    nc = _get_nc()
    in_maps = make_in_maps(x, Wq, Wk, Wv, Wo)
    res = run_bass_kernel_spmd(
        nc, in_maps, core_ids=list(range(8)), trace=trace, **spmd_kwargs
    )
    parts = [np.asarray(r["y"], dtype=np.float32) for r in res.results]
    y = np.stack([parts[2 * b] + parts[2 * b + 1] for b in range(B)])
    return y, res


def kernel(x, Wq, Wk, Wv, Wo):
    y, _ = run(x, Wq, Wk, Wv, Wo, trace=False)
    return y


# revision 29
# speedup vs baseline: 1.8938x; 1.8938x over previous
"""Trainium2 Bass kernel for nn_ChunkedAttention (B=4, T=4096, D=1024, H=16, dh=64).

Sharding: 8 cores = 4 batches x 2 head-groups (8 heads each). Host sums the
two partial out-projections per batch.

Datapath (per core):
  - Host passes x^T, so projections need no on-device transposes.
  - Q^T/K^T/V projected with fp32r matmuls, stored resident in SBUF as bf16.
  - Scores: per (qg, hp, kt) two K=64 bf16 matmuls packed into PE quadrants
    via tile_position -> PSUM f32 [k, q], one tile per head (j).
  - exp per (kt, j): j=0 exact Exp on ACT; j=1 Schraudolph fast-exp on DVE
    (i16 = rne(A*s + B), bitcast to bf16; ~1.8% rel RMS), with every 8th
    k-tile's j=1 on ACT to balance engine load. Per-j split halves the
    per-tile exp latency so the 2-ktile-deep AV software pipeline hides it.
  - AV: [dh+1, q] bf16 matmuls (lhsT = V with a ones column) accumulated
    over kt in PSUM; row 64 is the softmax denominator.
  - Normalize: copy av out of PSUM on ACT (frees the bank early), then
    reciprocal (DVE) -> DMA hop to partition 0 (partition_broadcast ucode
    reads p0) -> partition_broadcast (Pool) -> multiply (DVE). Head j=1's
    rows hop to partitions 64-127 via an SBUF->SBUF DMA.
  - Out-proj: K=128 bf16 matmuls over head-pair dims, accumulate over hp,
    yt staging copy on ACT, DMA -> y.
  - build_nc(repeat=n) emits the whole body n times in one NEFF; test.py
    uses the repeat-count slope to cancel dispatch overhead when timing.
"""

import os
import sys

import numpy as np

for _p in ("/opt/trn_rl_repo",):
    if _p not in sys.path and os.path.isdir(_p):
        sys.path.insert(0, _p)

import concourse.bass as bass
import concourse.mybir as mybir
import concourse.tile as tile
from concourse.bacc import Bacc
from concourse.bass_utils import run_bass_kernel_spmd

F32 = mybir.dt.float32
F32R = mybir.dt.float32r
BF16 = mybir.dt.bfloat16
I16 = mybir.dt.int16
EXP = mybir.ActivationFunctionType.Exp
MULT = mybir.AluOpType.mult
ADD = mybir.AluOpType.add

B, T, D = 4, 4096, 1024
HG = 512          # head-group width per core (8 heads x 64)
NH, DH = 8, 64    # heads per core, head dim
NPAIR = 4         # head pairs per core
QG = 512          # query-group width
NQG = T // QG     # 8
NKT = T // 128    # 32 k-tiles
NTC = T // 512    # 8 T-chunks in projection phase
NDC = D // 128    # 8 d_model chunks
SCALE = 1.0 / np.sqrt(DH)  # 0.125

# Schraudolph fast-exp in the bf16 bit domain: i16 = rne(A*logit + B16),
# bitcast i16 -> bf16 ~= exp(logit). Constants tuned for min RMS (~1.8%).
A16 = 184.6650390625            # 2**7 * log2(e)
B16 = 127.0 * 128.0 - 7.5
SCH_SCALE = A16 * SCALE         # folds the 1/sqrt(dh) into the affine


def build_nc(debug=None, repeat=1):
    nc = Bacc()
    xT_d = nc.dram_tensor("xT", [D, T], F32, kind="ExternalInput")
    wqT_d = nc.dram_tensor("wqT", [D, HG], F32, kind="ExternalInput")
    wkT_d = nc.dram_tensor("wkT", [D, HG], F32, kind="ExternalInput")
    wvT_d = nc.dram_tensor("wvT", [D, HG], F32, kind="ExternalInput")
    woT_d = nc.dram_tensor("woT", [HG, D], BF16, kind="ExternalInput")
    tri_d = nc.dram_tensor("tri", [128, 128], BF16, kind="ExternalInput")
    y_d = nc.dram_tensor("y", [T, D], F32, kind="ExternalOutput")

    with tile.TileContext(nc) as tc:
        with (
            tc.tile_pool(name="const", bufs=1) as pconst,
            tc.tile_pool(name="psS", bufs=4, space="PSUM") as psS,
            tc.tile_pool(name="psA", bufs=2, space="PSUM") as psA,
            tc.tile_pool(name="psO", bufs=2, space="PSUM") as psO,
        ):
            qt_sb = pconst.tile([128, NPAIR, T], BF16, tag="qt")
            kt_sb = pconst.tile([128, NPAIR, T], BF16, tag="kt")
            v_sb = pconst.tile([128, NKT, NH, DH + 1], BF16, tag="v")
            tri_sb = pconst.tile([128, 128], BF16, tag="tri")
            nc.sync.dma_start(tri_sb[:], tri_d[:])
            nc.gpsimd.memset(v_sb[:, :, :, DH : DH + 1], 1.0)

            for _rep in range(repeat):
                _emit_body(nc, tc, qt_sb, kt_sb, v_sb, tri_sb,
                           xT_d, wqT_d, wkT_d, wvT_d, woT_d, y_d,
                           psS, psA, psO)
    nc.compile()
    return nc


def _emit_body(nc, tc, qt_sb, kt_sb, v_sb, tri_sb,
               xT_d, wqT_d, wkT_d, wvT_d, woT_d, y_d, psS, psA, psO):
    if True:
        if True:
            # ---------- phase 1: Q^T/K^T/V projections from host-side x^T ----
            with (
                tc.tile_pool(name="pw", bufs=1) as pw,
                tc.tile_pool(name="pxin", bufs=2) as pxin,
            ):
                wq_sb = pw.tile([128, NDC, HG], F32R, tag="wq")
                wk_sb = pw.tile([128, NDC, HG], F32R, tag="wk")
                wv_sb = pw.tile([128, NDC, HG], F32R, tag="wv")
                xT_r = xT_d.bitcast(F32R).rearrange("(dc p) t -> p dc t", p=128)
                xt0 = pxin.tile([128, NDC, QG], F32R, tag="xt")
                for dc in range(NDC):
                    nc.sync.dma_start(xt0[:, dc, :], xT_r[:, dc, 0:QG])
                for dc in range(NDC):
                    nc.sync.dma_start(
                        wq_sb[:, dc, :],
                        wqT_d.bitcast(F32R).rearrange(
                            "(dc p) h -> p dc h", p=128
                        )[:, dc, :],
                    )
                for dc in range(NDC):
                    nc.sync.dma_start(
                        wk_sb[:, dc, :],
                        wkT_d.bitcast(F32R).rearrange(
                            "(dc p) h -> p dc h", p=128
                        )[:, dc, :],
                    )
                for dc in range(NDC):
                    nc.sync.dma_start(
                        wv_sb[:, dc, :],
                        wvT_d.bitcast(F32R).rearrange(
                            "(dc p) h -> p dc h", p=128
                        )[:, dc, :],
                    )
                for tcn in range(NTC):
                    if tcn == 0:
                        xt = xt0
                    else:
                        xt = pxin.tile([128, NDC, QG], F32R, tag="xt")
                        for dc in range(NDC):
                            nc.sync.dma_start(
                                xt[:, dc, :],
                                xT_r[:, dc, tcn * QG : (tcn + 1) * QG],
                            )
                    # Q^T, K^T -> [hg, t] bf16 resident
                    for hp in range(NPAIR):
                        pq = psS.tile([128, QG], F32, tag="s", name="pq")
                        for dc in range(NDC):
                            nc.tensor.matmul(
                                pq[:],
                                wq_sb[:, dc, hp * 128 : (hp + 1) * 128],
                                xt[:, dc, :],
                                start=(dc == 0),
                                stop=(dc == NDC - 1),
                            )
                        nc.vector.tensor_copy(
                            qt_sb[:, hp, tcn * QG : (tcn + 1) * QG], pq[:]
                        )
                        pk = psS.tile([128, QG], F32, tag="s", name="pk")
                        for dc in range(NDC):
                            nc.tensor.matmul(
                                pk[:],
                                wk_sb[:, dc, hp * 128 : (hp + 1) * 128],
                                xt[:, dc, :],
                                start=(dc == 0),
                                stop=(dc == NDC - 1),
                            )
                        nc.vector.tensor_copy(
                            kt_sb[:, hp, tcn * QG : (tcn + 1) * QG], pk[:]
                        )
                    # V -> [t, h, dh] bf16 resident (ones column at dh)
                    for ts in range(4):
                        pv = psO.tile([128, QG], F32, tag="op")
                        for dc in range(NDC):
                            nc.tensor.matmul(
                                pv[:],
                                xt[:, dc, ts * 128 : (ts + 1) * 128],
                                wv_sb[:, dc, :],
                                start=(dc == 0),
                                stop=(dc == NDC - 1),
                            )
                        nc.scalar.copy(
                            v_sb[:, tcn * 4 + ts, :, 0:DH],
                            pv.rearrange("p (h d) -> p h d", h=NH),
                        )

            # ---------- phase 2: attention + out-projection ----------
            with (
                tc.tile_pool(name="p2", bufs=1) as p2,
                tc.tile_pool(name="pet", bufs=6) as pet,
                tc.tile_pool(name="pnm", bufs=4) as pnm,
                tc.tile_pool(name="prc", bufs=3) as prc,
                tc.tile_pool(name="pmg", bufs=3) as pmg,
                tc.tile_pool(name="pyt", bufs=3) as pyt,
                tc.tile_pool(name="podt", bufs=3) as podt,
            ):
                wo_sb = p2.tile([128, NPAIR, D], BF16, tag="wo")
                nc.sync.dma_start(
                    wo_sb[:], woT_d.rearrange("(hp p) e -> p hp e", p=128)
                )
                def emit_outproj(oqg, omrgT, part):
                    # part 0..3 -> (half, qc-pair) chunk of the 8 op tiles
                    half = part // 2
                    for qc in (2 * (part % 2), 2 * (part % 2) + 1):
                        op = psO.tile([128, QG], F32, tag="op")
                        for hp in range(NPAIR):
                            nc.tensor.matmul(
                                op[:],
                                omrgT[:, hp, qc * 128 : (qc + 1) * 128],
                                wo_sb[:, hp, half * QG : (half + 1) * QG],
                                start=(hp == 0),
                                stop=(hp == NPAIR - 1),
                            )
                        yt = pyt.tile([128, QG], F32, tag="yt")
                        (nc.scalar.copy if qc % 2 == 0 else nc.vector.tensor_copy)(
                            yt[:], op[:]
                        )
                        nc.sync.dma_start(
                            y_d[
                                oqg * QG + qc * 128 : oqg * QG + (qc + 1) * 128,
                                half * QG : (half + 1) * QG,
                            ],
                            yt[:],
                        )

                prev = None
                for qg in range(NQG):
                    ktmax = 4 * (qg + 1)
                    mrgT = pmg.tile([128, NPAIR, QG], BF16, tag="mrgT")
                    for hp in range(NPAIR):
                        if prev is not None:
                            emit_outproj(prev[0], prev[1], hp)
                        avs = [
                            psA.tile([128, QG], F32, tag="av", name=f"av{j}")
                            for j in range(2)
                        ]

                        def emit_av(kt, dlt, e_t):
                            # V cols [data(64), ones] -> rows 0-63 data, 64 den
                            for j in range(2):
                                nc.tensor.matmul(
                                    avs[j][0 : DH + 1, dlt:],
                                    v_sb[:, kt, 2 * hp + j, :],
                                    e_t[:, j, dlt:],
                                    start=(kt == 0),
                                    stop=(kt == ktmax - 1),
                                )

                        pending = []
                        for kt in range(ktmax):
                            diag = kt - 4 * qg
                            dlt = 128 * diag if diag > 0 else 0
                            e_t = pet.tile([128, 2, QG], BF16, tag="e")
                            for j in range(2):
                                s_t = psS.tile(
                                    [128, QG], F32, tag="s", name=f"s{j}"
                                )
                                nc.tensor.matmul(
                                    s_t[:, dlt:],
                                    kt_sb[
                                        64 * j : 64 * (j + 1),
                                        hp,
                                        kt * 128 : (kt + 1) * 128,
                                    ],
                                    qt_sb[
                                        64 * j : 64 * (j + 1),
                                        hp,
                                        qg * QG + dlt : (qg + 1) * QG,
                                    ],
                                    start=True,
                                    stop=True,
                                    tile_position=(64 * j, 0),
                                )
                                # j=0 -> exact exp on ACT; j=1 -> fast exp on
                                # DVE/Pool (alternating by kt)
                                if j == 0:
                                    nc.scalar.activation(
                                        e_t[:, 0, dlt:],
                                        s_t[:, dlt:],
                                        EXP,
                                        scale=SCALE,
                                    )
                                elif kt % 8 == 7:
                                    nc.scalar.activation(
                                        e_t[:, 1, dlt:],
                                        s_t[:, dlt:],
                                        EXP,
                                        scale=SCALE,
                                    )
                                else:
                                    nc.vector.tensor_scalar(
                                        e_t.bitcast(I16)[:, 1, dlt:],
                                        s_t[:, dlt:],
                                        SCH_SCALE,
                                        B16,
                                        MULT,
                                        ADD,
                                    )
                                if diag >= 0:
                                    nc.vector.tensor_tensor(
                                        e_t[:, j, dlt : dlt + 128],
                                        e_t[:, j, dlt : dlt + 128],
                                        tri_sb[:],
                                        MULT,
                                    )
                            pending.append((kt, dlt, e_t))
                            if len(pending) > 2:
                                emit_av(*pending.pop(0))
                        while pending:
                            emit_av(*pending.pop(0))

                        # normalize: copy av out of PSUM fast (ACT) so the
                        # banks free early; reciprocal both dens into one row
                        # tile; one DMA hop to partition 0 (partition_broadcast
                        # ucode reads p0) + one broadcast covers both heads.
                        craws = []
                        rr = prc.tile([128, 2, QG], F32, tag="rr")
                        for j in range(2):
                            craw = prc.tile(
                                [128, QG], F32, tag="craw", name=f"craw{j}"
                            )
                            nc.scalar.copy(
                                craw[0 : DH + 1, :], avs[j][0 : DH + 1, :]
                            )
                            craws.append(craw)
                            nc.vector.reciprocal(
                                rr[DH : DH + 1, j, :], craw[DH : DH + 1, :]
                            )
                        rs0 = prc.tile([1, 2, QG], F32, tag="rs0")
                        nc.sync.dma_start(rs0[:], rr[DH : DH + 1, :, :])
                        rb = pnm.tile([64, 2, QG], F32, tag="rb")
                        nc.gpsimd.partition_broadcast(rb[:], rs0[:])
                        nc.vector.tensor_tensor(
                            mrgT[0:DH, hp, :],
                            craws[0][0:DH, :],
                            rb[:, 0, :],
                            MULT,
                        )
                        odt = podt.tile([64, QG], BF16, tag="odt")
                        nc.vector.tensor_tensor(
                            odt[:], craws[1][0:DH, :], rb[:, 1, :], MULT
                        )
                        nc.sync.dma_start(mrgT[DH:128, hp, :], odt[:])

                    prev = (qg, mrgT)
                for part in range(4):
                    emit_outproj(prev[0], prev[1], part)


_NC_CACHE = None


def _get_nc():
    global _NC_CACHE
    if _NC_CACHE is None:
        _NC_CACHE = build_nc()
    return _NC_CACHE


def make_in_maps(x, Wq, Wk, Wv, Wo):
    import ml_dtypes

    bf = ml_dtypes.bfloat16
    x = np.asarray(x, dtype=np.float32)
    Wq = np.asarray(Wq, dtype=np.float32)
    Wk = np.asarray(Wk, dtype=np.float32)
    Wv = np.asarray(Wv, dtype=np.float32)
    Wo = np.asarray(Wo, dtype=np.float32)
    tri = np.triu(np.ones((128, 128), dtype=np.float32)).astype(bf)
    in_maps = []
    for c in range(8):
        b, g = divmod(c, 2)
        rows = slice(HG * g, HG * (g + 1))
        in_maps.append(
            {
                "xT": np.ascontiguousarray(x[b].T),
                "wqT": np.ascontiguousarray(Wq[rows].T),
                "wkT": np.ascontiguousarray(Wk[rows].T),
                "wvT": np.ascontiguousarray(Wv[rows].T),
                "woT": np.ascontiguousarray(Wo[:, rows].T).astype(bf),
                "tri": tri,
            }
        )
    return in_maps


def run(x, Wq, Wk, Wv, Wo, trace=False, **spmd_kwargs):
    nc = _get_nc()
    in_maps = make_in_maps(x, Wq, Wk, Wv, Wo)
    res = run_bass_kernel_spmd(
        nc, in_maps, core_ids=list(range(8)), trace=trace, **spmd_kwargs
    )
    parts = [np.asarray(r["y"], dtype=np.float32) for r in res.results]
    y = np.stack([parts[2 * b] + parts[2 * b + 1] for b in range(B)])
    return y, res


def kernel(x, Wq, Wk, Wv, Wo):
    y, _ = run(x, Wq, Wk, Wv, Wo, trace=False)
    return y
